# revision 53
# baseline (speedup 1.0000x reference)
"""MLA (multi-head latent attention) prefill kernel for 8 TRN2 NeuronCores.

Sharding:
 - kv_a projection is data-parallel over the sequence (each core computes the
   latent + roped k_pe for its 512 positions from its x shard), then the
   UNNORMALIZED latent + k_pe + rms scale g are AllGathered in two stages
   (half the latent rides in gather-a so it can start early; the second half,
   k_pe, and g (bf16) ride in gather-b).
 - q is computed DIRECTLY head-sharded: each core projects its 2 heads over
   the FULL sequence from the full xT (no AllToAll).
 - kv_b expansion + attention + output projection are tensor-parallel over
   heads; each core produces a partial wo output (fp16) and the host sums the
   8 partials.

Device layout notes (all matmuls bf16, fp32 PSUM accumulation):
 - x is transposed on the host (xT) so projections run channel-major with
   weights as the stationary matmul operand.
 - Per-head qk channel order is [rope_lo(32); rope_hi(32); nope(64)] with the
   rope pairs deinterleaved on the host (wq / wkv_a rows permuted). RoPE then
   only combines partition ranges [0:32] x [32:64] straight out of PSUM.
 - Scores are computed k-major: st[k, q] = (k_tile)^T q, two k-tiles per PSUM
   group so one ACT Exp call covers [128, 1024]. The additive causal mask for
   boundary blocks is a CONSTANT SBUF tile (4 patterns, one per 128-offset)
   accumulated via an identity-weight matmul - no mask DMA.
 - Softmax denominator: the two probs tiles of a group are pair-summed on the
   DVE (bf16 2x) and a single ones-column matmul per pair accumulates the
   denominator, halving the PE streaming cost of the reduction. 1/den uses
   reciprocal_approx_fast, is broadcast across partitions with a rank-1
   matmul, and multiplies the unnormalized attention output.
 - rms scale g = exp(-0.5*ln(ms)) so Ln/Exp share one ACT table set.
 - attention output is produced transposed [dv, q]; wo consumes it directly
   and the partial output is written [m, s] in fp16; host transposes once.
"""

import os
import sys

sys.path.insert(0, "/opt/trn_rl_repo")

import numpy as np
import ml_dtypes

import concourse.bass as bass
import concourse.tile as tile
import concourse.mybir as mybir
from concourse import bacc
from concourse.bass_utils import run_bass_kernel_spmd
from concourse.masks import make_identity

BF16 = mybir.dt.bfloat16
F16 = mybir.dt.float16
F32 = mybir.dt.float32
NPBF16 = ml_dtypes.bfloat16

S = 4096          # sequence length
D = 2048          # model dim
H = 16            # total heads
HPC = 2           # heads per core
NCORES = 8
L = 1024          # kv lora rank
LH = L // 2
ROPE = 64
NOPE = 64
VH = 128          # v head dim
SCALE = 128.0 ** -0.5
EPS = 1e-6

SB = 512          # free-dim block size
NSB = S // SB     # 8
NE = D // 128     # 16 e-chunks
NL = L // 128     # 8 latent chunks
NKT = S // 128    # 32 k tiles
NDIAG = SB // 128  # 4 diag-offset mask patterns

last_results = None   # BassKernelResults of the most recent run (for test.py)

_BUILD_CACHE: dict = {}


def _bcast128(tensor, offset, n):
    """stride-0 partition-broadcast AP over a DRAM row."""
    return bass.AP(tensor=tensor, offset=offset, ap=[[0, 128], [1, n]])


def _build(skip, add, n_mconst):
    """Build + schedule the per-core Bass program.

    skip/add: [NKT][NSB] bool grids over (k-tile, q-block) mask blocks.
    n_mconst: number of distinct additive-mask patterns (0 if mask-free).
    """
    nc = bacc.Bacc("TRN2", target_bir_lowering=False, debug=False,
                   num_devices=NCORES)

    groups = [list(range(NCORES))]

    wqT_d = nc.dram_tensor("wqT", [D, 128 * HPC], BF16, kind="ExternalInput")
    wkvaT_d = nc.dram_tensor("wkvaT", [D, L + ROPE], BF16, kind="ExternalInput")
    wkvbk_d = nc.dram_tensor("wkvbTk", [L, NOPE * HPC], BF16, kind="ExternalInput")
    wkvbv_d = nc.dram_tensor("wkvbTv", [L, VH * HPC], BF16, kind="ExternalInput")
    woT_d = nc.dram_tensor("woT", [VH * HPC, D], BF16, kind="ExternalInput")
    xT_d = nc.dram_tensor("xT", [D, S], BF16, kind="ExternalInput")
    cosT_d = nc.dram_tensor("cosT", [32, S], F32, kind="ExternalInput")
    sinT_d = nc.dram_tensor("sinT", [32, S], F32, kind="ExternalInput")
    if n_mconst:
        mconst_d = nc.dram_tensor("mconst", [n_mconst, 128, SB], BF16,
                                  kind="ExternalInput")
    out_d = nc.dram_tensor("out", [D, S], F16, kind="ExternalOutput")

    xT_r = xT_d[:].rearrange("(eo p) s -> p eo s", p=128)

    with tile.TileContext(nc) as tc:
        with (
            tc.tile_pool(name="singles", bufs=1) as singles,
            tc.tile_pool(name="persist", bufs=1) as persist,
            tc.tile_pool(name="qbx", bufs=2) as qbx,
        ):
            ones_c = singles.tile([128, 1], BF16)
            nc.vector.memset(ones_c[:], 1.0)
            eps_t = singles.tile([1, 1], F32)
            nc.vector.memset(eps_t[:], EPS)
            wkvbk_s = singles.tile([128, NL, NOPE * HPC], BF16)
            wkvbv_s = singles.tile([128, NL, VH * HPC], BF16)
            wo_s = singles.tile([128, HPC, D], BF16)
            if n_mconst:
                mconst_s = singles.tile([128, n_mconst, SB], BF16)
                nc.gpsimd.dma_start(
                    mconst_s[:],
                    mconst_d[:].rearrange("j p s -> p j s"))



            def load_late_weights():
                nc.sync.dma_start(wkvbk_s[:], wkvbk_d[:].rearrange("(lo p) c -> p lo c", p=128))
                nc.sync.dma_start(wkvbv_s[:], wkvbv_d[:].rearrange("(lo p) c -> p lo c", p=128))
                nc.sync.dma_start(wo_s[:], woT_d[:].rearrange("(co p) m -> p co m", p=128))

            # warm the PE's HAM clock gate before the first real matmuls
            wtmp = singles.tile([128, 256], BF16)
            nc.vector.memset(wtmp[:], 0.0)
            with tc.tile_pool(name="pswarm", bufs=1, space="PSUM") as pswarm:
                wps = pswarm.tile([1, 256], F32)
                for wi in range(10):
                    nc.tensor.matmul(wps[:], ones_c[:], wtmp[:],
                                     start=True, stop=True)

            k0 = persist.tile([128, S], BF16)
            k1 = persist.tile([128, S], BF16)
            v_sb = persist.tile([128, NKT, VH * HPC], BF16)  # s-major v
            q_all = persist.tile([128, HPC, S], BF16)
            attn_T = persist.tile([128, HPC, S], BF16)

            # wq / wkv_a weights live only through the projection phase
            from contextlib import ExitStack
            _wk = ExitStack()
            wkvap = _wk.enter_context(tc.tile_pool(name="wkvap", bufs=1))
            wkva_s = wkvap.tile([128, NE, L + ROPE], BF16)
            wq_s = wkvap.tile([128, NE, 128 * HPC], BF16)
            wkva_r = wkvaT_d[:].rearrange("(eo p) c -> p eo c", p=128)
            wq_r = wqT_d[:].rearrange("(eo p) c -> p eo c", p=128)

            def mk_rope(rpool):
                def rope(ps_pe, cos_t, sin_t, out_lo, out_hi):
                    m1 = rpool.tile([32, SB], F32, tag="m1")
                    m2 = rpool.tile([32, SB], F32, tag="m2")
                    m3 = rpool.tile([32, SB], F32, tag="m3")
                    m4 = rpool.tile([32, SB], F32, tag="m4")
                    nc.vector.tensor_mul(m1[:], ps_pe[0:32, :], cos_t[:])
                    nc.vector.tensor_mul(m2[:], ps_pe[32:64, :], sin_t[:])
                    nc.vector.tensor_mul(m3[:], ps_pe[0:32, :], sin_t[:])
                    nc.vector.tensor_mul(m4[:], ps_pe[32:64, :], cos_t[:])
                    nc.vector.tensor_sub(out_lo, m1[:], m2[:])
                    nc.vector.tensor_add(out_hi, m3[:], m4[:])
                return rope

            # ---- phase A: own-shard kv_a + rms + two-piece AllGather -------
            # piece A: first GA_LT latent chunks - SMALL so its doorbell rings
            # early on every core (the collective rendezvous runs ~30us after
            # the last core's doorbell; a small piece A pulls that forward).
            # piece B rows: remaining latent, roped k_pe, rms scale g (bf16).
            GA_LT = 1
            GBL = NL - GA_LT             # latent chunks in piece B
            NGA = GA_LT * 128
            NGB = GBL * 128 + ROPE + 1
            gata_sh_d = nc.dram_tensor("gata_sh", [NGA, SB], BF16)
            gatb_sh_d = nc.dram_tensor("gatb_sh", [NGB, SB], BF16)
            gata_full = nc.dram_tensor("gata_full", [NCORES, NGA, SB],
                                       BF16, addr_space="Shared")
            gatb_full = nc.dram_tensor("gatb_full", [NCORES, NGB, SB],
                                       BF16, addr_space="Shared")
            with (
                tc.tile_pool(name="shx", bufs=1) as shx,
                tc.tile_pool(name="shw", bufs=1) as shw,
                tc.tile_pool(name="shr", bufs=1) as shr,
                tc.tile_pool(name="psS", bufs=6, space="PSUM") as psS,
            ):
                rope_s = mk_rope(shr)
                # own-shard x: loaded via shard-indexed AP built host-side
                xs_t = shx.tile([128, NE, SB], BF16, tag="xs")
                xsh_d = nc.dram_tensor("xTs", [D, SB], BF16, kind="ExternalInput")
                xs_r = xsh_d[:].rearrange("(eo p) s -> p eo s", p=128)
                for e in range(NE):
                    nc.sync.dma_start(wkva_s[:, e, :], wkva_r[:, e, :])
                    nc.scalar.dma_start(xs_t[:, e, :], xs_r[:, e, :])
                # wq early so q-proj can start the moment phase A drains; on
                # the sync queue these land behind the wkva/x chunks but ahead
                # of the x stream.
                for e in range(NE):
                    nc.sync.dma_start(wq_s[:, e, :], wq_r[:, e, :])
                load_late_weights()
                cosS_d = nc.dram_tensor("cosS", [32, SB], F32, kind="ExternalInput")
                sinS_d = nc.dram_tensor("sinS", [32, SB], F32, kind="ExternalInput")
                cos_s = shw.tile([32, SB], F32, tag="coss")
                nc.gpsimd.dma_start(cos_s[:], cosS_d[:])
                sin_s = shw.tile([32, SB], F32, tag="sins")
                nc.gpsimd.dma_start(sin_s[:], sinS_d[:])

                sq_t = shx.tile([128, NL, SB], BF16, tag="sq")
                lat_t = shx.tile([128, NL, SB], BF16, tag="lat")

                # latent stays UNNORMALIZED; g rides in gather-b and is
                # applied to the kv_b outputs on the consumer side.
                # e-outer so the half finishes right after the last x chunk
                # lands instead of replaying the chunk sequence per lt.
                def lat_group(lts):
                    lps = [psS.tile([128, SB], F32, tag="ps", name=f"lp{lt}")
                           for lt in lts]
                    for e in range(NE):
                        for i, lt in enumerate(lts):
                            nc.tensor.matmul(lps[i][:],
                                             wkva_s[:, e, lt * 128:(lt + 1) * 128],
                                             xs_t[:, e, :],
                                             start=(e == 0), stop=(e == NE - 1))
                    for i, lt in enumerate(lts):
                        nc.scalar.activation(sq_t[:, lt, :], lps[i][:],
                                             mybir.ActivationFunctionType.Square)
                        nc.vector.tensor_copy(lat_t[:, lt, :], lps[i][:])

                lat_group(range(GA_LT))
                # stores ride the scalar DMA queue so the big sync-queue x
                # stream is never head-of-line blocked behind them
                nc.scalar.dma_start(
                    gata_sh_d[:].rearrange("(lt p) s -> p lt s", p=128),
                    lat_t[:, 0:GA_LT, :])
                nc.gpsimd.collective_compute(
                    "AllGather", mybir.AluOpType.bypass,
                    replica_groups=groups,
                    ins=[gata_sh_d[:]], outs=[gata_full[:]],
                )
                lat_group(range(GA_LT, NL))
                nc.scalar.dma_start(
                    gatb_sh_d[0:GBL * 128, :].rearrange("(lt p) s -> p lt s", p=128),
                    lat_t[:, GA_LT:NL, :])

                # k_pe
                kp = psS.tile([64, SB], F32, tag="ps")
                for e in range(NE):
                    nc.tensor.matmul(kp[:], wkva_s[:, e, L:L + ROPE],
                                     xs_t[:, e, :],
                                     start=(e == 0), stop=(e == NE - 1))
                kpe_t = shw.tile([64, SB], BF16, tag="kpe")
                rope_s(kp, cos_s, sin_s, kpe_t[0:32, :], kpe_t[32:64, :])
                nc.scalar.dma_start(gatb_sh_d[GBL * 128:GBL * 128 + ROPE, :],
                                    kpe_t[:])

                # rms scale g = exp(-0.5 * ln(mean(lat^2) + eps)); Ln and Exp
                # share one ACT table set.
                sp = psS.tile([1, SB], F32, tag="ps")
                for lt in range(NL):
                    nc.tensor.matmul(sp[:], ones_c[:], sq_t[:, lt, :],
                                     start=(lt == 0), stop=(lt == NL - 1))
                ln_t = shw.tile([1, SB], F32, tag="lnms")
                nc.scalar.activation(ln_t[:], sp[:],
                                     mybir.ActivationFunctionType.Ln,
                                     bias=eps_t[:], scale=1.0 / L)
                g_t = shw.tile([1, SB], BF16, tag="g")
                nc.scalar.activation(g_t[:], ln_t[:],
                                     mybir.ActivationFunctionType.Exp,
                                     scale=-0.5)
                nc.scalar.dma_start(gatb_sh_d[NGB - 1:NGB, :], g_t[:])
                nc.gpsimd.collective_compute(
                    "AllGather", mybir.AluOpType.bypass,
                    replica_groups=groups,
                    ins=[gatb_sh_d[:]], outs=[gatb_full[:]],
                )

            # ---- phase B: q projection, own 2 heads over the full seq ------
            with (
                tc.tile_pool(name="qbw", bufs=2) as qbw,
                tc.tile_pool(name="qrp", bufs=2) as qrp,
                tc.tile_pool(name="psQ", bufs=4, space="PSUM") as psQ,
            ):
                rope_q = mk_rope(qrp)
                for sb2 in range(NSB):
                    ssl = slice(sb2 * SB, (sb2 + 1) * SB)
                    x_t = qbx.tile([128, NE, SB], BF16, tag="x")
                    nc.sync.dma_start(x_t[:], xT_r[:, :, ssl])
                    cos_t = qbw.tile([32, SB], F32, tag="cos")
                    nc.gpsimd.dma_start(cos_t[:], cosT_d[:, ssl])
                    sin_t = qbw.tile([32, SB], F32, tag="sin")
                    nc.gpsimd.dma_start(sin_t[:], sinT_d[:, ssl])
                    for ct in range(HPC):
                        qp = psQ.tile([128, SB], F32, tag="ps")
                        for e in range(NE):
                            nc.tensor.matmul(qp[:], wq_s[:, e, ct * 128:(ct + 1) * 128],
                                             x_t[:, e, :], start=(e == 0), stop=(e == NE - 1))
                        nc.scalar.copy(q_all[64:128, ct, ssl], qp[64:128, :])
                        rope_q(qp, cos_t, sin_t,
                               q_all[0:32, ct, ssl], q_all[32:64, ct, ssl])

            _wk.close()  # release wq/wkva space

            # ------- phase C: kv_b interleaved with attention ---------------
            with (
                tc.tile_pool(name="lg2", bufs=2) as lg2,
                tc.tile_pool(name="prb", bufs=8) as prb,
                tc.tile_pool(name="prp", bufs=8) as prp,
                tc.tile_pool(name="dvp", bufs=2) as dvp,
                tc.tile_pool(name="ost", bufs=3) as ost,
                tc.tile_pool(name="psatt", bufs=2, space="PSUM") as psatt,
                tc.tile_pool(name="psden", bufs=2, space="PSUM") as psden,
                tc.tile_pool(name="pso", bufs=2, space="PSUM") as pso,
            ):
                def kvb(sb):
                    ssl = slice(sb * SB, (sb + 1) * SB)
                    # unnormalized latent + roped k_pe + g from the AllGathers
                    lg_t = lg2.tile([128, NL, SB], BF16, tag="lat")
                    nc.sync.dma_start(
                        lg_t[:, 0:GA_LT, :],
                        gata_full[sb, :, :].rearrange("(lt p) s -> p lt s", p=128))
                    nc.sync.dma_start(
                        lg_t[:, GA_LT:NL, :],
                        gatb_full[sb, 0:GBL * 128, :].rearrange("(lt p) s -> p lt s", p=128))
                    nc.sync.dma_start(k0[0:64, ssl],
                                      gatb_full[sb, GBL * 128:GBL * 128 + ROPE, :])
                    nc.vector.tensor_copy(k1[0:64, ssl], k0[0:64, ssl])
                    g_off = (sb * NGB + NGB - 1) * SB
                    gb_t = lg2.tile([128, SB], BF16, tag="gb")
                    nc.gpsimd.dma_start(gb_t[:], _bcast128(gatb_full, g_off, SB))
                    gc_b = lg2.tile([128, SB // 128], BF16, tag="gcb")
                    for st in range(SB // 128):
                        nc.gpsimd.dma_start(
                            gc_b[:, st:st + 1],
                            gatb_full[sb, NGB - 1:NGB,
                                      st * 128:(st + 1) * 128].rearrange("o p -> p o"))
                    gc_t = lg2.tile([128, SB // 128], F32, tag="gc")
                    nc.vector.tensor_copy(gc_t[:], gc_b[:])

                    kbp = psatt.tile([128, SB], F32, tag="att")
                    for lt in range(NL):
                        nc.tensor.matmul(kbp[:], wkvbk_s[:, lt, :], lg_t[:, lt, :],
                                         start=(lt == 0), stop=(lt == NL - 1))
                    nc.vector.tensor_mul(k0[64:128, ssl], kbp[0:64, :], gb_t[64:128, :])
                    nc.vector.tensor_mul(k1[64:128, ssl], kbp[64:128, :], gb_t[64:128, :])

                    for st in range(SB // 128):
                        vp = psatt.tile([128, VH * HPC], F32, tag="att")
                        for lt in range(NL):
                            nc.tensor.matmul(vp[:],
                                             lg_t[:, lt, st * 128:(st + 1) * 128],
                                             wkvbv_s[:, lt, :],
                                             start=(lt == 0), stop=(lt == NL - 1))
                        nc.vector.tensor_scalar_mul(v_sb[:, sb * 4 + st, :], vp[:],
                                                    gc_t[:, st:st + 1])

                def attn(qb):
                    qsl = slice(qb * SB, (qb + 1) * SB)
                    active = [ki for ki in range(NKT) if not skip[ki][qb]]
                    pairs = [active[i:i + 2] for i in range(0, len(active), 2)]
                    npair = len(pairs)
                    # heads interleave and den/AV lag the scores by ~2 pairs:
                    # the in-order PE queue then never waits on the Exp ->
                    # mask-mul -> pair-add chain of the same pair
                    nquad = (npair + 1) // 2
                    noct = (nquad + 1) // 2
                    dp = [psden.tile([1, SB], F32, tag="den", name=f"dp{h}")
                          for h in range(HPC)]
                    op_ = [pso.tile([128, SB], F32, tag="o", name=f"op{h}")
                           for h in range(HPC)]
                    pend = []       # deferred AV matmuls
                    dend = []       # deferred oct-den matmuls
                    half_pair = [None, None]  # per-head stashed (pi, pair_t)
                    half_quad = [None, None]  # per-head stashed (qi, quad_t)

                    def put_quad(qi, h, quad_t, last):
                        # fold two quads into an oct before the ones-matmul
                        if half_quad[h] is None and not last:
                            half_quad[h] = (qi, quad_t)
                            return
                        if half_quad[h] is None:
                            dend.append((qi // 2, h, quad_t))
                        else:
                            q0, prev_q = half_quad[h]
                            oct_t = prp.tile([128, SB], BF16, tag="oct")
                            nc.vector.tensor_add(oct_t[:], prev_q[:], quad_t[:])
                            dend.append((qi // 2, h, oct_t))
                            half_quad[h] = None

                    def flush(n):
                        while len(dend) > n // 4:
                            oi, h, oct_t = dend.pop(0)
                            nc.tensor.matmul(dp[h][:], ones_c[:], oct_t[:],
                                             start=(oi == 0), stop=(oi == noct - 1))
                        while len(pend) > n:
                            pi, h, pr, pg = pend.pop(0)
                            for gi, ki in enumerate(pr):
                                nc.tensor.matmul(op_[h][:],
                                                 v_sb[:, ki, h * VH:(h + 1) * VH],
                                                 pg[:, gi, :],
                                                 start=(pi == 0 and gi == 0),
                                                 stop=(pi == npair - 1 and gi == len(pr) - 1))

                    for pi, pr in enumerate(pairs):
                        for h in range(HPC):
                            kh = k0 if h == 0 else k1
                            ps = psatt.tile([128, 2, SB], F32, tag="att")
                            pg = prb.tile([128, 2, SB], BF16, tag="pg")
                            for gi, ki in enumerate(pr):
                                nc.tensor.matmul(ps[:, gi, :],
                                                 kh[:, ki * 128:(ki + 1) * 128],
                                                 q_all[:, h, qsl],
                                                 start=True, stop=True)
                            if len(pr) == 2:
                                nc.scalar.activation(pg[:, :, :], ps[:, :, :],
                                                     mybir.ActivationFunctionType.Exp,
                                                     scale=SCALE)
                            else:
                                nc.scalar.activation(pg[:, 0, :], ps[:, 0, :],
                                                     mybir.ActivationFunctionType.Exp,
                                                     scale=SCALE)
                            # causal boundary: multiplicative 0/1 mask on DVE
                            for gi, ki in enumerate(pr):
                                am = add[ki][qb]
                                if am >= 0:
                                    nc.vector.tensor_mul(pg[:, gi, :], pg[:, gi, :],
                                                         mconst_s[:, am, :])
                            pair_t = prp.tile([128, SB], BF16, tag="pair")
                            if len(pr) == 2:
                                nc.vector.tensor_add(pair_t[:], pg[:, 0, :], pg[:, 1, :])
                            else:
                                nc.vector.tensor_copy(pair_t[:], pg[:, 0, :])
                            # pair-sums fold into quads then octs on the DVE,
                            # so the ones-matmul reduction streams 1/4 as much
                            last = pi == npair - 1
                            if half_pair[h] is None and not last:
                                half_pair[h] = (pi, pair_t)
                            elif half_pair[h] is None:
                                put_quad(pi // 2, h, pair_t, True)
                            else:
                                p0, prev_t = half_pair[h]
                                quad_t = prp.tile([128, SB], BF16, tag="quad")
                                nc.vector.tensor_add(quad_t[:], prev_t[:], pair_t[:])
                                put_quad(pi // 2, h, quad_t, last)
                                half_pair[h] = None
                            pend.append((pi, h, pr, pg))
                            flush(6)
                    flush(0)
                    for h in range(HPC):
                        dinv_f = dvp.tile([1, SB], F32, tag="dinvf")
                        nc.vector.reciprocal_approx_fast(dinv_f[:], dp[h][:])
                        dbs = dvp.tile([128, SB], F32, tag="dbs")
                        nc.gpsimd.partition_broadcast(dbs[:], dinv_f[:])
                        nc.vector.tensor_mul(attn_T[:, h, qsl], op_[h][:], dbs[:])

                # kv_b for all s-blocks back to back (dense PE work while the
                # attention pipeline warms), then attention in causal order -
                # attn(qb)'s k/v prefix is ready by the time the PE gets there
                for sb in range(NSB):
                    kvb(sb)
                for qb in range(NSB):
                    if any(not skip[ki][qb] for ki in range(NKT)):
                        attn(qb)
                    else:
                        nc.vector.memset(attn_T[:, :, qb * SB:(qb + 1) * SB], 0.0)

                # ---------------- phase D: output projection -----------------
                # s2 outer so each s-block's wo work is ready as soon as its
                # attention blocks finish -> overlaps the attention tail.
                for s2 in range(NSB):
                    for mt in range(D // 128):
                        wp = psatt.tile([128, SB], F32, tag="att")
                        for cc in range(HPC):
                            nc.tensor.matmul(wp[:], wo_s[:, cc, mt * 128:(mt + 1) * 128],
                                             attn_T[:, cc, s2 * SB:(s2 + 1) * SB],
                                             start=(cc == 0), stop=(cc == HPC - 1))
                        o_t = ost.tile([128, SB], F16, tag="ostage")
                        nc.vector.tensor_copy(o_t[:], wp[:])
                        nc.sync.dma_start(
                            out_d[mt * 128:(mt + 1) * 128, s2 * SB:(s2 + 1) * SB],
                            o_t[:])

    nc.compile()
    return nc


def kernel(x, cos, sin, mask, wq, wkv_a, kv_norm_w, wkv_b, wo, start_pos=0):
    x = np.asarray(x, np.float32)
    cos = np.asarray(cos, np.float32)
    sin = np.asarray(sin, np.float32)
    mask = np.asarray(mask, np.float32)
    wq = np.asarray(wq, np.float32)
    wkv_a = np.asarray(wkv_a, np.float32)
    kv_norm_w = np.asarray(kv_norm_w, np.float32)
    wkv_b = np.asarray(wkv_b, np.float32)
    wo = np.asarray(wo, np.float32)

    # mask block metadata: [qb, qi, kt, kj]
    mr = mask.reshape(NSB, SB, NKT, 128)
    skip_qk = (mr <= -1e8).all(axis=(1, 3))          # [qb, kt]
    nonzero_qk = (mr != 0).any(axis=(1, 3))          # [qb, kt]
    skip = skip_qk.T.copy()                          # [kt, qb]
    add_blocks = (nonzero_qk & ~skip_qk).T           # [kt, qb]

    # collect distinct additive-mask patterns; add[kt][qb] = pattern idx or -1
    patterns = []
    add = [[-1] * NSB for _ in range(NKT)]
    for kt in range(NKT):
        for qb in range(NSB):
            if not add_blocks[kt][qb]:
                continue
            # multiplicative 0/1 mask (applied to probs on the DVE); only
            # valid for pure -inf/0 masks, which is what a causal mask is
            raw = mask[qb * SB:(qb + 1) * SB, kt * 128:(kt + 1) * 128].T
            assert np.all((raw == 0.0) | (raw <= -1e8)), "non-causal mask"
            blk = np.ascontiguousarray((raw == 0.0).astype(np.float32)).astype(NPBF16)
            for i, p in enumerate(patterns):
                if np.array_equal(p, blk):
                    add[kt][qb] = i
                    break
            else:
                patterns.append(blk)
                add[kt][qb] = len(patterns) - 1
    n_mconst = len(patterns)

    key = (tuple(map(tuple, skip)), tuple(map(tuple, add)))
    if key not in _BUILD_CACHE:
        _BUILD_CACHE[key] = _build(skip, add, n_mconst)
    nc = _BUILD_CACHE[key]

    # ---- host-side shard prep ----
    deint = np.concatenate([np.arange(0, ROPE, 2), np.arange(1, ROPE, 2)])
    wq_h = wq.reshape(H, 128, D)
    # per-head row order [rope deinterleaved; nope]
    qrows = np.concatenate([wq_h[:, NOPE + deint, :], wq_h[:, 0:NOPE, :]], axis=1)
    wkva_perm = np.concatenate([wkv_a[0:L], wkv_a[L + deint]], axis=0)
    wkvb_h = wkv_b.reshape(H, NOPE + VH, L)

    xT = np.ascontiguousarray(x[0].T).astype(NPBF16)
    cosT = np.ascontiguousarray(cos.T)
    sinT = np.ascontiguousarray(sin.T)
    wkvaT = np.ascontiguousarray(wkva_perm.T).astype(NPBF16)
    shared = {"wkvaT": wkvaT, "xT": xT, "cosT": cosT, "sinT": sinT}
    if n_mconst:
        shared["mconst"] = np.stack(patterns)

    in_maps = []
    for c in range(NCORES):
        hs = [HPC * c + i for i in range(HPC)]
        k_rows = (wkvb_h[hs, 0:NOPE, :] * kv_norm_w[None, None, :]).reshape(
            NOPE * HPC, L)
        wkvbTk_c = np.ascontiguousarray(k_rows.T).astype(NPBF16)
        v_rows = wkvb_h[hs, NOPE:, :].reshape(VH * HPC, L)
        wkvbTv_c = np.ascontiguousarray(v_rows.T).astype(NPBF16)
        woT_c = np.ascontiguousarray(
            wo[:, hs[0] * VH:(hs[-1] + 1) * VH].T).astype(NPBF16)
        m = dict(shared)
        m.update({"wkvbTk": wkvbTk_c, "wkvbTv": wkvbTv_c, "woT": woT_c})
        ssl = slice(c * SB, (c + 1) * SB)
        m["xTs"] = np.ascontiguousarray(xT[:, ssl])
        m["cosS"] = np.ascontiguousarray(cosT[:, ssl])
        m["sinS"] = np.ascontiguousarray(sinT[:, ssl])
        m["wqT"] = np.ascontiguousarray(
            qrows[hs].reshape(128 * HPC, D).T).astype(NPBF16)
        in_maps.append(m)

    trace = os.environ.get("KERNEL_TRACE", "0") == "1"
    if trace:
        _install_ntff_hook()
    global last_results
    last_results = run_bass_kernel_spmd(nc, in_maps, core_ids=list(range(NCORES)),
                                        trace=trace)
    total = np.zeros((D, S), np.float32)
    for r in last_results.results:
        total += r["out"].astype(np.float32)
    return np.ascontiguousarray(total.T)[None]


def _install_ntff_hook():
    """Register the axon NTFF profiling hook (used when KERNEL_TRACE=1)."""
    import types
    import ctypes
    import contextlib

    if "antenv.axon_hooks" in sys.modules:
        return
    try:
        so = ctypes.CDLL("/opt/axon/libaxon_pjrt.so")
        so.axon_start_nrt_profile
    except (OSError, AttributeError):
        return
    so.axon_start_nrt_profile.argtypes = [ctypes.POINTER(ctypes.c_int64),
                                          ctypes.c_size_t]
    so.axon_start_nrt_profile.restype = ctypes.c_int64
    so.axon_stop_nrt_profile.argtypes = [ctypes.c_char_p]
    so.axon_stop_nrt_profile.restype = ctypes.c_int64

    @contextlib.contextmanager
    def _hook(output_dir, device_ids):
        import jax
        jax.devices()
        if device_ids:
            ids = (ctypes.c_int64 * len(device_ids))(*device_ids)
            rc = so.axon_start_nrt_profile(ids, len(device_ids))
        else:
            rc = so.axon_start_nrt_profile(None, 0)
        if rc != 0:
            raise RuntimeError(f"axon_start_nrt_profile rc={rc}")
        try:
            yield
        finally:
            n = so.axon_stop_nrt_profile(str(output_dir).encode())
            if n < 0:
                raise RuntimeError(f"axon_stop_nrt_profile rc={n}")

    mod = types.ModuleType("antenv.axon_hooks")
    mod.get_axon_ntff_profile_hook = lambda: _hook
    mod.set_axon_ntff_profile_hook = lambda h: None
    sys.modules["antenv.axon_hooks"] = mod
